# revision 9
# baseline (speedup 1.0000x reference)
"""nn_AttentionOut_63711544869147 — causal multi-head attention + output projection,
distributed over 8 Trainium2 NeuronCores.

Module: out = softmax(causal(Q K^T / sqrt(d))) V @ W_O + b_O, returned with the
(unchanged) residual: reference returns the tuple (residual, out).

Sharding (8 cores = 2 batches x 4 head-groups of 4 heads, SPMD single program):
  each core computes full causal attention for its batch over its 4 heads and
  a partial projection  sum_{h in group} z_h @ W_O[h]  ->  pout [2048, 1024].
  The host sums the 4 head-group partials per batch (the "all-reduce" of the
  row-sharded W_O product), adds b_O, and passes the residual through.

Device dataflow per (head-pair, 512-wide q strip), exact causal tiling. The two
heads of a pair (A at PE rows 0-63, B at rows 64-127) share each [128, 2, 512]
score tile — half 0 = head A, half 1 = head B for the SAME kv tile — so their
64-deep QK matmuls run concurrently in disjoint PE row groups and one
1024-wide exp serves both heads:
  scores_T[kv,q] = K_h^T_tile.T @ Q_h^T_strip           (PE, bf16)
  expP = exp(scores_T * 1/8)      (split between ACT exact-exp and DVE
                                   Schraudolph exp — bf16 bit-trick via a
                                   single tensor_scalar into int16 — to
                                   balance the two engines' queues)
  diagonal kv tiles: one mask multiply per tile over both straddle blocks
  (doubled-tri operand), run on the otherwise-idle GpSimd engine
  z_ext[65,q] += V_ext_tile.T @ expP                    (PE accum; V_ext = [V | 1]
                                                         so row 64 = softmax denom)
  PV matmuls are emitted L tiles behind QK/exp so a stalled PV (waiting for a
  z-bank recycle through the normalize chain) never head-of-line-blocks the
  next tiles' QK matmuls in the PE queue; the projection of strip s is
  likewise emitted inside strip s+1's first tile loop.
  z = z_ext[0:64] * (1/z_ext[64])  (DVE approx-reciprocal + DRAM-bounce
                                    broadcast DMA; fp32 denominators throughout)
  pout strip = z_T @ W_O_group                          (PE, 256-deep contraction)
  pout stored fp16, strip-row DMAs issued from the GpSimd queue; host
  accumulates the 4 partials per batch in fp32.
"""

import numpy as np

import concourse.bass as bass
import concourse.bacc as bacc
import concourse.tile as tile
from concourse import mybir
from concourse.bass_utils import run_bass_kernel_spmd

F32 = mybir.dt.float32
F16 = mybir.dt.float16
I16 = mybir.dt.int16
BF16 = mybir.dt.bfloat16

N_CORES = 8
N_HEADS = 16
H = 4          # heads per core
S = 2048
D = 64
P = 128
D_MODEL = 1024
NSTRIP = 4     # q strips of 512
QW = 512       # strip width
LOOKAHEAD = 6  # PV deferral depth (tiles)

# bf16-bit-trick exp: bits16 = rne(score * (0.125*2^7/ln2) + (127*2^7 - 7.25))
SCHR_A = float(0.125 * 128.0 / np.log(2.0))
SCHR_B = float(127 * 128 - 7.25)


def dve_exp(s, j, t):
    """Which tiles' exp runs on DVE (Schraudolph) instead of ACT (exact)."""
    return t % 4 == 1


def build_program():
    MMDT = BF16
    nc = bacc.Bacc(target_bir_lowering=False)

    qT = nc.dram_tensor("qT", [H, D, S], MMDT, kind="ExternalInput")
    kT = nc.dram_tensor("kT", [H, D, S], MMDT, kind="ExternalInput")
    vx = nc.dram_tensor("vx", [H, P, 16, D + 1], MMDT, kind="ExternalInput")
    wo = nc.dram_tensor("wo", [2 * P, D_MODEL], MMDT, kind="ExternalInput")
    tri2 = nc.dram_tensor("tri2", [P, 2 * P], MMDT, kind="ExternalInput")
    pout = nc.dram_tensor("pout", [S, D_MODEL], F16, kind="ExternalOutput")

    with tile.TileContext(nc) as tc:
        with (
            tc.tile_pool(name="persist", bufs=1) as persist,
            tc.tile_pool(name="expp", bufs=8) as expp,
            tc.tile_pool(name="rcpp", bufs=2) as rcpp,
            tc.tile_pool(name="rbp", bufs=2) as rbp,
            tc.tile_pool(name="outp", bufs=4) as outp,
            tc.tile_pool(name="znp", bufs=4) as znp,
            tc.tile_pool(name="scps", bufs=2, space="PSUM") as scps,
            tc.tile_pool(name="zps", bufs=2, space="PSUM") as zps,
            tc.tile_pool(name="wops", bufs=2, space="PSUM") as wops,
            tc.tile_pool(name="dramp", bufs=2, space="DRAM") as dramp,
        ):
            # ---- persistent loads (everything stays SBUF-resident) ----
            # ordered so strip 0 / head-pair 0 can start as early as possible
            qT_sb = [None, None]
            kT_sb = [None, None]
            wo_sb = [None, None]
            vext_sb = [None] * H

            # input loads spread across engine DMA queues for parallelism
            kT_sb[0] = persist.tile([P, S], MMDT, tag="kT0", name="kT0")
            nc.sync.dma_start(kT_sb[0][:], kT[0:2].rearrange("h d s -> (h d) s"))
            qT_sb[0] = persist.tile([P, S], MMDT, tag="qT0", name="qT0")
            nc.scalar.dma_start(qT_sb[0][:], qT[0:2].rearrange("h d s -> (h d) s"))
            tri_sb = persist.tile([P, 2, P], MMDT, tag="tri", name="tri_sb")
            nc.gpsimd.dma_start(tri_sb[:], tri2[:].rearrange("p (a b) -> p a b", a=2))
            for h in (0, 1):
                vext_sb[h] = persist.tile([P, 16, D + 1], MMDT, tag=f"vext{h}", name=f"vext{h}")
                nc.gpsimd.dma_start(vext_sb[h][:], vx[h])
            kT_sb[1] = persist.tile([P, S], MMDT, tag="kT1", name="kT1")
            nc.sync.dma_start(kT_sb[1][:], kT[2:4].rearrange("h d s -> (h d) s"))
            qT_sb[1] = persist.tile([P, S], MMDT, tag="qT1", name="qT1")
            nc.scalar.dma_start(qT_sb[1][:], qT[2:4].rearrange("h d s -> (h d) s"))
            for h in (2, 3):
                vext_sb[h] = persist.tile([P, 16, D + 1], MMDT, tag=f"vext{h}", name=f"vext{h}")
                nc.gpsimd.dma_start(vext_sb[h][:], vx[h])
            for j in range(2):
                wo_sb[j] = persist.tile([P, D_MODEL], MMDT, tag=f"wo{j}", name=f"wo{j}")
                nc.sync.dma_start(wo_sb[j][:], wo[P * j : P * (j + 1), :])

            pending_proj = [None]
            pending_tt = [None]

            def emit_pending_tt():
                """Deferred normalize multiplies: emitted inside the NEXT
                pair's tile loop so the DRAM-bounce latency never head-of-
                line-blocks the Vector queue's exp stream."""
                if pending_tt[0] is None:
                    return
                for zn_dst, z_src, rb in pending_tt[0]:
                    nc.vector.tensor_mul(zn_dst, z_src, rb)
                pending_tt[0] = None

            def emit_proj(s, zn_sb):
                for qb in range(4):
                    ops = [
                        wops.tile([P, 512], F32, tag="wo_ps", name=f"wo_ps{mt}")
                        for mt in range(2)
                    ]
                    for j2 in range(2):
                        for mt in range(2):
                            nc.tensor.matmul(
                                ops[mt][:],
                                zn_sb[j2][:, qb * P : (qb + 1) * P],
                                wo_sb[j2][:, mt * 512 : (mt + 1) * 512],
                                start=(j2 == 0),
                                stop=(j2 == 1),
                            )
                    ot = outp.tile([P, 2, 512], F16, tag="ot", name="ot")
                    for mt in range(2):
                        nc.vector.tensor_copy(ot[:, mt, :], ops[mt][:])
                    nc.gpsimd.dma_start(
                        pout[(4 * s + qb) * P : (4 * s + qb + 1) * P, :],
                        ot[:].rearrange("p a b -> p (a b)"),
                    )

            # ---- main loops ----
            for s in range(NSTRIP):
                q0 = s * QW
                nt = 4 * s + 4  # kv tiles; last four straddle the diagonal
                zn_sb = [znp.tile([P, QW], MMDT, tag=f"zn{j}", name=f"zn{j}") for j in range(2)]
                for j in range(2):
                    hA, hB = 2 * j, 2 * j + 1
                    z_ps = [
                        zps.tile([D + 1, QW], F32, tag="z", name=f"z{x}")
                        for x in ("A", "B")
                    ]
                    ex_q = {}

                    def emit_pv(t):
                        li = max(0, (t - 4 * s)) * P
                        ex = ex_q.pop(t)
                        for side, h in ((0, hA), (1, hB)):
                            nc.tensor.matmul(
                                z_ps[side][:, li:QW],
                                vext_sb[h][:, t, :],
                                ex[:, side, li:QW],
                                start=(t == 0),
                                stop=(t == nt - 1),
                            )

                    for t in range(nt):
                        li = max(0, (t - 4 * s)) * P  # partial range of diag tiles
                        sc = scps.tile([P, 2, QW], F32, tag="sc", name="sc")
                        # the two heads' 64-deep QK matmuls target disjoint PE
                        # row groups (rows 0-63 / 64-127) and run concurrently
                        for side in (0, 1):
                            off = side * D
                            nc.tensor.matmul(
                                sc[:, side, li:QW],
                                kT_sb[j][off : off + D, t * P : (t + 1) * P],
                                qT_sb[j][off : off + D, q0 + li : q0 + QW],
                                start=True,
                                stop=True,
                            )
                        ex = expp.tile([P, 2, QW], MMDT, tag="ex", name="ex")
                        if dve_exp(s, j, t):
                            nc.vector.tensor_scalar(
                                ex[:, :, li:QW].bitcast(I16), sc[:, :, li:QW],
                                SCHR_A, SCHR_B,
                                mybir.AluOpType.mult, mybir.AluOpType.add,
                            )
                        else:
                            nc.scalar.activation(
                                ex[:, :, li:QW], sc[:, :, li:QW],
                                mybir.ActivationFunctionType.Exp, scale=0.125,
                            )
                        if t >= 4 * s:
                            # one mask multiply over both heads' straddle blocks
                            m = ex[:, :, li : li + P]
                            nc.gpsimd.tensor_mul(m, m, tri_sb[:])
                        ex_q[t] = ex
                        if t >= LOOKAHEAD:
                            emit_pv(t - LOOKAHEAD)
                        if t == 2:
                            # deferred work from the previous pair/strip, far
                            # enough in to not stall any queue
                            emit_pending_tt()
                            if j == 0 and pending_proj[0] is not None:
                                emit_proj(*pending_proj[0])
                                pending_proj[0] = None
                    for t in range(max(0, nt - LOOKAHEAD), nt):
                        emit_pv(t)
                    # normalize: z[0:64] * (1 / z[64]); approx recip is ~5x
                    # faster than the 8-pass exact DVE reciprocal and exact to
                    # ~4e-6, far below the bf16 input rounding. The final
                    # multiplies are deferred (see emit_pending_tt); the dcp
                    # copies run on the Scalar queue to balance the engines.
                    tts = []
                    for side in (0, 1):
                        off = side * D
                        dcp = rcpp.tile([1, QW], F32, tag="dcp", name="dcp")
                        nc.scalar.copy(dcp[:], z_ps[side][D : D + 1, :])
                        rcp = rcpp.tile([1, QW], F32, tag="rcp", name="rcp")
                        # (custom-DVE op requires an SBUF input; PSUM reads garbage)
                        nc.vector.reciprocal_approx_fast(rcp[:], dcp[:])
                        # broadcast 1/denom across the 64 d-partitions via a
                        # DRAM bounce: DRAM sources allow a step-0 partition dim
                        rdr = dramp.tile([1, QW], F32, tag="rdr", name="rdr")
                        nc.sync.dma_start(rdr[:], rcp[:])
                        rb_sb = rbp.tile([D, QW], F32, tag="rb_sb", name="rb_sb")
                        nc.sync.dma_start(
                            rb_sb[:],
                            bass.AP(tensor=rdr.tensor, offset=rdr.offset,
                                    ap=[[0, D]] + [list(a) for a in rdr.ap][1:]),
                        )
                        tts.append(
                            (zn_sb[j][off : off + D, :], z_ps[side][0:D, :], rb_sb[:])
                        )
                    pending_tt[0] = tts
                # defer this strip's projection into the next strip's first
                # tile loop so it never blocks the next QK matmuls in the PE
                # queue (the last strip has nothing after it: emit now)
                pending_proj[0] = (s, zn_sb)
                if s == NSTRIP - 1:
                    emit_pending_tt()
                    emit_proj(*pending_proj[0])
                    pending_proj[0] = None

    nc.finalize()
    return nc


_PROGRAM = None
LAST_RESULTS = None


def _get_program():
    global _PROGRAM
    if _PROGRAM is None:
        _PROGRAM = build_program()
    return _PROGRAM


def make_in_maps(q, k, v, W_O, n_cores=N_CORES):
    """Shard full inputs into per-core maps (core = batch*4 + head_group)."""
    import ml_dtypes
    mmdt = ml_dtypes.bfloat16
    q = np.ascontiguousarray(np.asarray(q, dtype=np.float32))
    k = np.ascontiguousarray(np.asarray(k, dtype=np.float32))
    v = np.ascontiguousarray(np.asarray(v, dtype=np.float32))
    W_O = np.ascontiguousarray(np.asarray(W_O, dtype=np.float32))
    B = q.shape[0]
    qT = np.ascontiguousarray(q.reshape(B, S, N_HEADS, D).transpose(0, 2, 3, 1))
    kT = np.ascontiguousarray(k.reshape(B, S, N_HEADS, D).transpose(0, 2, 3, 1))
    # v extended with a ones column (softmax denominator row) and pre-arranged
    # to the on-chip [partition, kv_tile, d+1] layout so the DMA is contiguous
    vh = v.reshape(B, S, N_HEADS, D).transpose(0, 2, 1, 3)  # [B, H, S, D]
    vext = np.concatenate(
        [vh, np.ones((B, N_HEADS, S, 1), dtype=np.float32)], axis=3
    ).reshape(B, N_HEADS, 16, P, D + 1).transpose(0, 1, 3, 2, 4)  # [B, Hh, P, 16, D+1]
    # mask[kv, q] = 1 iff kv <= q  (scores live transposed: partition=kv, free=q)
    tri = np.triu(np.ones((P, P), dtype=np.float32))
    tri2 = np.ascontiguousarray(np.concatenate([tri, tri], axis=1))
    in_maps = []
    for core in range(n_cores):
        b, g = core // 4, core % 4
        hs = slice(H * g, H * (g + 1))
        in_maps.append(
            {
                "qT": np.ascontiguousarray(qT[b, hs]).astype(mmdt),
                "kT": np.ascontiguousarray(kT[b, hs]).astype(mmdt),
                "vx": np.ascontiguousarray(vext[b, hs]).astype(mmdt),
                "wo": np.ascontiguousarray(W_O[hs].reshape(2 * P, D_MODEL)).astype(mmdt),
                "tri2": tri2.astype(mmdt),
            }
        )
    return in_maps


def kernel(residual, q, k, v, W_O, b_O, _trace=False, _trace_kwargs=None):
    global LAST_RESULTS
    residual = np.asarray(residual, dtype=np.float32)
    B = residual.shape[0]
    in_maps = make_in_maps(q, k, v, W_O)
    nc = _get_program()
    res = run_bass_kernel_spmd(
        nc, in_maps, list(range(N_CORES)), trace=_trace, **(_trace_kwargs or {})
    )
    LAST_RESULTS = res
    out = np.zeros((B, S, D_MODEL), dtype=np.float32)
    for core in range(N_CORES):
        out[core // 4] += res.results[core]["pout"].astype(np.float32)
    out += np.asarray(b_O, dtype=np.float32)
    return (residual, out.astype(np.float32))
